# revision 5
# baseline (speedup 1.0000x reference)
"""Batched complex DFT (x @ W, N=256) via radix-2 DIF split, fp16 end-to-end,
data-parallel across 8 Trainium2 NeuronCores.

Math (decimation in frequency): with a = x_lo + x_hi, d = x_lo - x_hi
(column halves of x), the provided DFT matrix W satisfies
  X[:, 2m]   = (a @ W128)[:, m]          W128[n, m] = W[n, 2m]   (n, m < 128)
  X[:, 2m+1] = (d @ W_O)[:, m]           W_O = diag(W[1, :128]) @ W128
so the 256-point DFT becomes two 128-contraction complex matmuls — half the
MACs of the direct form, at full 128-partition PE efficiency.

Per core (shard = 32768 rows), per 128-row tile:
  - GpSimd computes the 'a' butterfly, DVE the 'd' butterfly (2D-contiguous
    fp16 operands via a host-fused [lo_r|lo_i|hi_r|hi_i] input layout, so
    the DVE 2x 16-bit mode applies).
  - PE transposes a_r/a_i/d_r/d_i as fp16 [128,128] blocks into PSUM
    (fp16 transpose streams 1 row/cycle vs fp32's 2); DVE evacuates two
    tiles' transposes per COPY to amortize fixed costs.
  - 4 fp16 matmuls (k=128, moving dim 256) accumulate even/odd complex
    outputs into fp32 PSUM; the Activation engine casts two tiles per
    ACTIVATE to fp16 staging; 1/sqrt(N) is folded into the W packs.
  - Host does only dtype casts + input column grouping + the output
    even/odd interleave; fp16 I/O halves HBM traffic (the baseline
    bottleneck). absmax rel err ~5e-4, well under the 2e-2 gate.

Inputs stream on the SP HWDGE ring, outputs on the ACT ring; per-partition
DMA descriptors are 8 KiB contiguous via the (n p t) row permutation
(identical on input and output, so it cancels).
"""

import numpy as np

P = 128
N = 256
NCORES = 8
B = 262144
M = B // NCORES            # 32768 rows per core
T = 16                     # 128-row tiles per DMA block
BLOCKS = M // (P * T)      # 16

_CACHE = {}


def _build():
    if "nc" in _CACHE:
        return _CACHE["nc"]

    import concourse.mybir as mybir
    import concourse.tile as tile
    from concourse import bacc
    from concourse.masks import make_identity

    F16 = mybir.dt.float16
    F32 = mybir.dt.float32
    W = 2 * N  # 512: fused row width (lo_r | lo_i | hi_r | hi_i)

    nc = bacc.Bacc("TRN2", debug=False, target_bir_lowering=False)

    x = nc.dram_tensor("x", [M, W], F16, kind="ExternalInput").ap()
    w1 = nc.dram_tensor("w1", [P, N], F16, kind="ExternalInput").ap()
    w2 = nc.dram_tensor("w2", [P, N], F16, kind="ExternalInput").ap()
    w1o = nc.dram_tensor("w1o", [P, N], F16, kind="ExternalInput").ap()
    w2o = nc.dram_tensor("w2o", [P, N], F16, kind="ExternalInput").ap()
    y = nc.dram_tensor("y", [M, W], F16, kind="ExternalOutput").ap()

    # Partition p holds T consecutive DRAM rows -> 8 KiB of contiguous DRAM
    # per partition per half-block DMA. Same permutation on input and
    # output, so it cancels.
    x_t = x.rearrange("(n p t) k -> n p t k", p=P, t=T)
    y_t = y.rearrange("(n p t) k -> n p t k", p=P, t=T)

    with tile.TileContext(nc) as tc:
        with (
            tc.tile_pool(name="consts", bufs=1) as consts,
            tc.tile_pool(name="xin", bufs=3) as xin_pool,
            tc.tile_pool(name="bf", bufs=4) as bf_pool,
            tc.tile_pool(name="xt", bufs=3) as xt_pool,
            tc.tile_pool(name="stage", bufs=2) as stage_pool,
            tc.tile_pool(name="pst", bufs=2, space="PSUM") as pst_pool,
            tc.tile_pool(name="pso", bufs=2, space="PSUM") as pso_pool,
        ):
            ident = consts.tile([P, P], F16)
            make_identity(nc, ident)

            w1_sb = consts.tile([P, N], F16)
            w2_sb = consts.tile([P, N], F16)
            w1o_sb = consts.tile([P, N], F16)
            w2o_sb = consts.tile([P, N], F16)
            nc.sync.dma_start(w1_sb, w1)
            nc.sync.dma_start(w2_sb, w2)
            nc.sync.dma_start(w1o_sb, w1o)
            nc.sync.dma_start(w2o_sb, w2o)

            h = T // 2
            for n in range(BLOCKS):
                xin = xin_pool.tile([P, T, W], F16, tag="xin")
                nc.sync.dma_start(xin[:, 0:h], x_t[n, :, 0:h])
                nc.sync.dma_start(xin[:, h:T], x_t[n, :, h:T])
                stage = stage_pool.tile([P, T, W], F16, tag="st")
                for tp in range(T // 2):
                    # two 128-row tiles share one PSUM-evac COPY and one
                    # output ACTIVATE to amortize fixed per-op costs
                    psT = pst_pool.tile([P, 2, 4 * P], F16, tag="pt")
                    ps = pso_pool.tile([P, 2, W], F32, tag="po")
                    for j in range(2):
                        t = 2 * tp + j
                        # butterfly: bf = [ a_r | a_i | d_r | d_i ]
                        bf = bf_pool.tile([P, 4 * P], F16, tag="bf")
                        nc.gpsimd.tensor_tensor(
                            bf[:, 0 : 2 * P], xin[:, t, 0 : 2 * P],
                            xin[:, t, 2 * P : 4 * P], mybir.AluOpType.add,
                        )
                        nc.vector.tensor_tensor(
                            bf[:, 2 * P : 4 * P], xin[:, t, 0 : 2 * P],
                            xin[:, t, 2 * P : 4 * P], mybir.AluOpType.subtract,
                        )
                        # psT[j]: [ T(a_r) | T(a_i) | T(d_r) | T(d_i) ]
                        nc.tensor.transpose(psT[:, j, 0 * P : 1 * P], bf[:, 0 * P : 1 * P], ident)
                        nc.tensor.transpose(psT[:, j, 1 * P : 2 * P], bf[:, 1 * P : 2 * P], ident)
                        nc.tensor.transpose(psT[:, j, 2 * P : 3 * P], bf[:, 2 * P : 3 * P], ident)
                        nc.tensor.transpose(psT[:, j, 3 * P : 4 * P], bf[:, 3 * P : 4 * P], ident)
                    xt = xt_pool.tile([P, 2, 4 * P], F16, tag="xt")
                    nc.vector.tensor_copy(xt, psT)
                    for j in range(2):
                        # ps[j]: [ X_even(re|im) | X_odd(re|im) ]
                        nc.tensor.matmul(ps[:, j, 0:N], xt[:, j, 0 * P : 1 * P], w1_sb, start=True, stop=False)
                        nc.tensor.matmul(ps[:, j, 0:N], xt[:, j, 1 * P : 2 * P], w2_sb, start=False, stop=True)
                        nc.tensor.matmul(ps[:, j, N : 2 * N], xt[:, j, 2 * P : 3 * P], w1o_sb, start=True, stop=False)
                        nc.tensor.matmul(ps[:, j, N : 2 * N], xt[:, j, 3 * P : 4 * P], w2o_sb, start=False, stop=True)
                    nc.scalar.copy(stage[:, 2 * tp : 2 * tp + 2], ps)
                nc.scalar.dma_start(y_t[n, :, 0:h], stage[:, 0:h])
                nc.scalar.dma_start(y_t[n, :, h:T], stage[:, h:T])

    nc.compile()
    _CACHE["nc"] = nc
    return nc


def _make_w_packs(W_real, W_imag):
    Wc = W_real.astype(np.float64) + 1j * W_imag.astype(np.float64)
    W128 = Wc[:P, 0:N:2]                  # W128[n, m] = W[n, 2m]
    W_O = Wc[1, :P][:, None] * W128       # twiddle fold: diag(W[1, :128]) @ W128
    s = 1.0 / np.sqrt(N)
    packs = []
    for Wm in (W128, W_O):
        re = (np.real(Wm) * s).astype(np.float16)
        im = (np.imag(Wm) * s).astype(np.float16)
        packs.append(np.ascontiguousarray(np.concatenate([re, im], axis=1)))
        packs.append(np.ascontiguousarray(np.concatenate([-im, re], axis=1)))
    return packs  # w1, w2, w1o, w2o


def kernel(x_real, x_imag, W_real, W_imag):
    from concourse.bass_utils import run_bass_kernel_spmd

    x_real = np.asarray(x_real, dtype=np.float32)
    x_imag = np.asarray(x_imag, dtype=np.float32)
    assert x_real.shape == (B, N) and x_imag.shape == (B, N)

    # fused fp16 input rows: [ lo_r | lo_i | hi_r | hi_i ]
    x_dev = np.empty((B, 2 * N), dtype=np.float16)
    x_dev[:, 0:128] = x_real[:, 0:128]
    x_dev[:, 128:256] = x_imag[:, 0:128]
    x_dev[:, 256:384] = x_real[:, 128:256]
    x_dev[:, 384:512] = x_imag[:, 128:256]
    w1, w2, w1o, w2o = _make_w_packs(np.asarray(W_real), np.asarray(W_imag))

    nc = _build()

    in_maps = [
        {
            "x": x_dev[i * M : (i + 1) * M],
            "w1": w1,
            "w2": w2,
            "w1o": w1o,
            "w2o": w2o,
        }
        for i in range(NCORES)
    ]
    res = run_bass_kernel_spmd(nc, in_maps, core_ids=list(range(NCORES)))
    yfull = np.concatenate([r["y"] for r in res.results], axis=0)  # [B, 512] f16

    real = np.empty((B, N), dtype=np.float32)
    imag = np.empty((B, N), dtype=np.float32)
    real[:, 0::2] = yfull[:, 0:128]        # X_even re
    imag[:, 0::2] = yfull[:, 128:256]      # X_even im
    real[:, 1::2] = yfull[:, 256:384]      # X_odd re
    imag[:, 1::2] = yfull[:, 384:512]      # X_odd im
    return real, imag


# revision 6
# speedup vs baseline: 1.0914x; 1.0914x over previous
"""Batched complex DFT (x @ W, N=256) via radix-2 DIF split, fp16 end-to-end,
data-parallel across 8 Trainium2 NeuronCores.

Math (decimation in frequency): with a = x_lo + x_hi, d = x_lo - x_hi
(column halves of x), the provided DFT matrix W satisfies
  X[:, 2m]   = (a @ W128)[:, m]          W128[n, m] = W[n, 2m]   (n, m < 128)
  X[:, 2m+1] = (d @ W_O)[:, m]           W_O = diag(W[1, :128]) @ W128
so the 256-point DFT becomes two 128-contraction complex matmuls — half the
MACs of the direct form, at full 128-partition PE efficiency.

The host ships x PRE-TRANSPOSED (contraction dim on partitions, a pure
layout permutation + fp16 cast), which removes the PE transposes AND the
PSUM->SBUF evacuation copy entirely. Per core (shard = 32768 rows), per
pair of 128-row tiles:
  - GpSimd computes the 'a' butterfly, DVE the 'd' butterfly (one paired
    512-elem fp16 op each), writing the matmul stationaries directly.
  - 8 fp16 matmuls (k=128, moving dim 256) accumulate even/odd complex
    outputs into fp32 PSUM; the Activation engine casts both tiles with
    one ACTIVATE to fp16 staging; 1/sqrt(N) is folded into the W packs.
  - Host does dtype casts, the input transpose, and the output even/odd
    interleave; fp16 I/O halves HBM traffic vs the fp32 baseline.
    absmax rel err ~5e-4, well under the 2e-2 gate.

Inputs stream on the SP HWDGE ring (4 KiB/partition descriptors), outputs
on the ACT ring (1 KiB descriptors, row t*128+p order).
"""

import numpy as np

P = 128
N = 256
NCORES = 8
B = 262144
M = B // NCORES            # 32768 rows per core
T = 16                     # 128-row tiles per DMA block
BLOCKS = M // (P * T)      # 16

_CACHE = {}


def _build():
    if "nc" in _CACHE:
        return _CACHE["nc"]

    import concourse.mybir as mybir
    import concourse.tile as tile
    from concourse import bacc

    F16 = mybir.dt.float16
    F32 = mybir.dt.float32
    W = 2 * N  # 512

    nc = bacc.Bacc("TRN2", debug=False, target_bir_lowering=False)

    # x transposed on host: [4 groups (lo_r, lo_i, hi_r, hi_i), 128 k, M rows]
    x = nc.dram_tensor("x", [4, P, M], F16, kind="ExternalInput").ap()
    w1 = nc.dram_tensor("w1", [P, N], F16, kind="ExternalInput").ap()
    w2 = nc.dram_tensor("w2", [P, N], F16, kind="ExternalInput").ap()
    w1o = nc.dram_tensor("w1o", [P, N], F16, kind="ExternalInput").ap()
    w2o = nc.dram_tensor("w2o", [P, N], F16, kind="ExternalInput").ap()
    y = nc.dram_tensor("y", [M, W], F16, kind="ExternalOutput").ap()

    C = T * P  # 2048 row-columns per block
    x_t = x.rearrange("g p (n c) -> n p g c", c=C)
    # psum partition p of tile t holds row n*2048 + t*128 + p
    y_t = y.rearrange("(n t p) k -> n p t k", t=T, p=P)

    with tile.TileContext(nc) as tc:
        with (
            tc.tile_pool(name="consts", bufs=1) as consts,
            tc.tile_pool(name="xin", bufs=3) as xin_pool,
            tc.tile_pool(name="xt", bufs=3) as xt_pool,
            tc.tile_pool(name="stage", bufs=2) as stage_pool,
            tc.tile_pool(name="pso", bufs=3, space="PSUM") as pso_pool,
        ):
            w1_sb = consts.tile([P, N], F16)
            w2_sb = consts.tile([P, N], F16)
            w1o_sb = consts.tile([P, N], F16)
            w2o_sb = consts.tile([P, N], F16)
            nc.sync.dma_start(w1_sb, w1)
            nc.sync.dma_start(w2_sb, w2)
            nc.sync.dma_start(w1o_sb, w1o)
            nc.sync.dma_start(w2o_sb, w2o)

            h = T // 2
            for n in range(BLOCKS):
                xin = xin_pool.tile([P, 4, C], F16, tag="xin")
                nc.sync.dma_start(xin[:, 0:2], x_t[n, :, 0:2])
                nc.sync.dma_start(xin[:, 2:4], x_t[n, :, 2:4])
                stage = stage_pool.tile([P, T, W], F16, tag="st")
                for tp in range(T // 2):
                    # two 128-row tiles per butterfly op / output ACTIVATE
                    s = slice(tp * 2 * P, (tp + 1) * 2 * P)
                    # xt: [ a_r | a_i | d_r | d_i ] x 256 row-columns
                    xt = xt_pool.tile([P, 4, 2 * P], F16, tag="xt")
                    nc.gpsimd.tensor_tensor(
                        xt[:, 0:2], xin[:, 0:2, s], xin[:, 2:4, s],
                        mybir.AluOpType.add,
                    )
                    nc.vector.tensor_tensor(
                        xt[:, 2:4], xin[:, 0:2, s], xin[:, 2:4, s],
                        mybir.AluOpType.subtract,
                    )
                    # ps[j]: [ X_even(re|im) | X_odd(re|im) ]
                    ps = pso_pool.tile([P, 2, W], F32, tag="po")
                    for j in range(2):
                        jj = slice(j * P, (j + 1) * P)
                        nc.tensor.matmul(ps[:, j, 0:N], xt[:, 0, jj], w1_sb, start=True, stop=False)
                        nc.tensor.matmul(ps[:, j, 0:N], xt[:, 1, jj], w2_sb, start=False, stop=True)
                        nc.tensor.matmul(ps[:, j, N:W], xt[:, 2, jj], w1o_sb, start=True, stop=False)
                        nc.tensor.matmul(ps[:, j, N:W], xt[:, 3, jj], w2o_sb, start=False, stop=True)
                    nc.scalar.copy(stage[:, 2 * tp : 2 * tp + 2], ps)
                nc.scalar.dma_start(y_t[n, :, 0:h], stage[:, 0:h])
                nc.scalar.dma_start(y_t[n, :, h:T], stage[:, h:T])

    nc.compile()
    _CACHE["nc"] = nc
    return nc


def _make_w_packs(W_real, W_imag):
    Wc = W_real.astype(np.float64) + 1j * W_imag.astype(np.float64)
    W128 = Wc[:P, 0:N:2]                  # W128[n, m] = W[n, 2m]
    W_O = Wc[1, :P][:, None] * W128       # twiddle fold: diag(W[1, :128]) @ W128
    s = 1.0 / np.sqrt(N)
    packs = []
    for Wm in (W128, W_O):
        re = (np.real(Wm) * s).astype(np.float16)
        im = (np.imag(Wm) * s).astype(np.float16)
        packs.append(np.ascontiguousarray(np.concatenate([re, im], axis=1)))
        packs.append(np.ascontiguousarray(np.concatenate([-im, re], axis=1)))
    return packs  # w1, w2, w1o, w2o


def kernel(x_real, x_imag, W_real, W_imag):
    from concourse.bass_utils import run_bass_kernel_spmd

    x_real = np.asarray(x_real, dtype=np.float32)
    x_imag = np.asarray(x_imag, dtype=np.float32)
    assert x_real.shape == (B, N) and x_imag.shape == (B, N)

    # fp16 cast (fast, SIMD), then per-shard blocked transposes
    xr16 = x_real.astype(np.float16)
    xi16 = x_imag.astype(np.float16)
    w1, w2, w1o, w2o = _make_w_packs(np.asarray(W_real), np.asarray(W_imag))

    nc = _build()

    in_maps = []
    for i in range(NCORES):
        sl = slice(i * M, (i + 1) * M)
        xd = np.empty((4, P, M), dtype=np.float16)
        xd[0] = xr16[sl, 0:128].T
        xd[1] = xi16[sl, 0:128].T
        xd[2] = xr16[sl, 128:256].T
        xd[3] = xi16[sl, 128:256].T
        in_maps.append({"x": xd, "w1": w1, "w2": w2, "w1o": w1o, "w2o": w2o})
    res = run_bass_kernel_spmd(nc, in_maps, core_ids=list(range(NCORES)))
    yfull = np.concatenate([r["y"] for r in res.results], axis=0)  # [B, 512] f16

    real = np.empty((B, N), dtype=np.float32)
    imag = np.empty((B, N), dtype=np.float32)
    real[:, 0::2] = yfull[:, 0:128]        # X_even re
    imag[:, 0::2] = yfull[:, 128:256]      # X_even im
    real[:, 1::2] = yfull[:, 256:384]      # X_odd re
    imag[:, 1::2] = yfull[:, 384:512]      # X_odd im
    return real, imag


# revision 10
# speedup vs baseline: 1.1956x; 1.0955x over previous
"""Batched complex DFT (x @ W, N=256) via radix-2 DIF split, fp16 end-to-end,
data-parallel across 8 Trainium2 NeuronCores.

Math (decimation in frequency): with a = x_lo + x_hi, d = x_lo - x_hi
(column halves of x), the provided DFT matrix W satisfies
  X[:, 2m]   = (a @ W128)[:, m]          W128[n, m] = W[n, 2m]   (n, m < 128)
  X[:, 2m+1] = (d @ W_O)[:, m]           W_O = diag(W[1, :128]) @ W128
so the 256-point DFT becomes two 128-contraction complex matmuls — half the
MACs of the direct form, at full 128-partition PE efficiency.

The host ships x PRE-TRANSPOSED (contraction dim on partitions, a pure
layout permutation + fp16 cast), which removes the PE transposes AND the
PSUM->SBUF evacuation copy entirely. Per core (shard = 32768 rows), per
pair of 128-row tiles:
  - GpSimd computes the 'a' butterfly, DVE the 'd' butterfly (one paired
    512-elem fp16 op each), writing the matmul stationaries directly.
  - 8 fp16 matmuls (k=128, moving dim 256) accumulate even/odd complex
    outputs into fp32 PSUM; the Activation engine casts both tiles with
    one ACTIVATE to fp16 staging; 1/sqrt(N) is folded into the W packs.
  - Host does dtype casts, the input transpose, and the output even/odd
    interleave; fp16 I/O halves HBM traffic vs the fp32 baseline.
    absmax rel err ~5e-4, well under the 2e-2 gate.

Inputs stream on the SP HWDGE ring (4 KiB/partition descriptors), outputs
on the ACT ring (1 KiB descriptors, row t*128+p order).
"""

import numpy as np

P = 128
N = 256
NCORES = 8
B = 262144
M = B // NCORES            # 32768 rows per core
T = 16                     # 128-row tiles per DMA block
BLOCKS = M // (P * T)      # 16

_CACHE = {}


def _build():
    if "nc" in _CACHE:
        return _CACHE["nc"]

    import concourse.mybir as mybir
    import concourse.tile as tile
    from concourse import bacc

    F16 = mybir.dt.float16
    F32 = mybir.dt.float32
    W = 2 * N  # 512

    nc = bacc.Bacc("TRN2", debug=False, target_bir_lowering=False)

    C = T * P  # 2048 row-columns per block
    # x transposed + block-permuted on host:
    # [h (lo/hi), 128 k, block, two (re/im), C] -> 8 KiB contiguous DRAM per
    # partition per half-input DMA
    x = nc.dram_tensor("x", [2, P, BLOCKS, 2, C], F16, kind="ExternalInput").ap()
    w1 = nc.dram_tensor("w1", [P, N], F16, kind="ExternalInput").ap()
    w2 = nc.dram_tensor("w2", [P, N], F16, kind="ExternalInput").ap()
    w1o = nc.dram_tensor("w1o", [P, N], F16, kind="ExternalInput").ap()
    w2o = nc.dram_tensor("w2o", [P, N], F16, kind="ExternalInput").ap()
    y = nc.dram_tensor("y", [M, W], F16, kind="ExternalOutput").ap()

    x_t = x.rearrange("h p n two c -> n p h two c")
    # device column n*2048 + t*128 + q holds original row n*2048 + q*16 + t
    # (host-side permutation), so partition q writes 16 consecutive DRAM rows
    y_t = y.rearrange("(n p t) k -> n p t k", p=P, t=T)

    with tile.TileContext(nc) as tc:
        with (
            tc.tile_pool(name="consts", bufs=1) as consts,
            tc.tile_pool(name="xin", bufs=3) as xin_pool,
            tc.tile_pool(name="xt", bufs=4) as xt_pool,
            tc.tile_pool(name="stage", bufs=3) as stage_pool,
            tc.tile_pool(name="pso", bufs=3, space="PSUM") as pso_pool,
        ):
            w1_sb = consts.tile([P, N], F16)
            w2_sb = consts.tile([P, N], F16)
            w1o_sb = consts.tile([P, N], F16)
            w2o_sb = consts.tile([P, N], F16)
            nc.sync.dma_start(w1_sb, w1)
            nc.sync.dma_start(w2_sb, w2)
            nc.sync.dma_start(w1o_sb, w1o)
            nc.sync.dma_start(w2o_sb, w2o)

            h = T // 2
            for n in range(BLOCKS):
                xin = xin_pool.tile([P, 2, 2, C], F16, tag="xin")
                nc.sync.dma_start(xin[:, 0], x_t[n, :, 0])
                nc.sync.dma_start(xin[:, 1], x_t[n, :, 1])
                stage = stage_pool.tile([P, T, W], F16, tag="st")
                for tp in range(T // 2):
                    # two 128-row tiles per butterfly op / output ACTIVATE
                    s = slice(tp * 2 * P, (tp + 1) * 2 * P)
                    # xt: [ a_r | a_i | d_r | d_i ] x 256 row-columns
                    xt = xt_pool.tile([P, 4, 2 * P], F16, tag="xt")
                    nc.gpsimd.tensor_tensor(
                        xt[:, 0:2], xin[:, 0, :, s], xin[:, 1, :, s],
                        mybir.AluOpType.add,
                    )
                    nc.vector.tensor_tensor(
                        xt[:, 2:4], xin[:, 0, :, s], xin[:, 1, :, s],
                        mybir.AluOpType.subtract,
                    )
                    # ps[j]: [ X_even(re|im) | X_odd(re|im) ]
                    ps = pso_pool.tile([P, 2, W], F32, tag="po")
                    for j in range(2):
                        jj = slice(j * P, (j + 1) * P)
                        nc.tensor.matmul(ps[:, j, 0:N], xt[:, 0, jj], w1_sb, start=True, stop=False)
                        nc.tensor.matmul(ps[:, j, 0:N], xt[:, 1, jj], w2_sb, start=False, stop=True)
                        nc.tensor.matmul(ps[:, j, N:W], xt[:, 2, jj], w1o_sb, start=True, stop=False)
                        nc.tensor.matmul(ps[:, j, N:W], xt[:, 3, jj], w2o_sb, start=False, stop=True)
                    nc.scalar.copy(stage[:, 2 * tp : 2 * tp + 2], ps)
                nc.scalar.dma_start(y_t[n, :, 0:h], stage[:, 0:h])
                nc.scalar.dma_start(y_t[n, :, h:T], stage[:, h:T])

    nc.compile()
    _CACHE["nc"] = nc
    return nc


def _make_w_packs(W_real, W_imag):
    Wc = W_real.astype(np.float64) + 1j * W_imag.astype(np.float64)
    W128 = Wc[:P, 0:N:2]                  # W128[n, m] = W[n, 2m]
    W_O = Wc[1, :P][:, None] * W128       # twiddle fold: diag(W[1, :128]) @ W128
    s = 1.0 / np.sqrt(N)
    packs = []
    for Wm in (W128, W_O):
        re = (np.real(Wm) * s).astype(np.float16)
        im = (np.imag(Wm) * s).astype(np.float16)
        packs.append(np.ascontiguousarray(np.concatenate([re, im], axis=1)))
        packs.append(np.ascontiguousarray(np.concatenate([-im, re], axis=1)))
    return packs  # w1, w2, w1o, w2o


def kernel(x_real, x_imag, W_real, W_imag):
    from concourse.bass_utils import run_bass_kernel_spmd

    x_real = np.asarray(x_real, dtype=np.float32)
    x_imag = np.asarray(x_imag, dtype=np.float32)
    assert x_real.shape == (B, N) and x_imag.shape == (B, N)

    # fp16 cast (fast, SIMD), then per-shard permuted transposes
    xr16 = x_real.astype(np.float16)
    xi16 = x_imag.astype(np.float16)
    w1, w2, w1o, w2o = _make_w_packs(np.asarray(W_real), np.asarray(W_imag))

    nc = _build()

    C = T * P
    # device column (n, t, q) <- original row (n, q, t): output partition q
    # then holds 16 consecutive DRAM rows per block (big DMA descriptors)
    pi = np.arange(M).reshape(BLOCKS, P, T).transpose(0, 2, 1).reshape(M)

    in_maps = []
    for i in range(NCORES):
        sl = slice(i * M, (i + 1) * M)
        xp_r = xr16[sl][pi]
        xp_i = xi16[sl][pi]
        xd = np.empty((2, P, BLOCKS, 2, C), dtype=np.float16)
        xd[0, :, :, 0] = xp_r[:, 0:128].T.reshape(P, BLOCKS, C)
        xd[0, :, :, 1] = xp_i[:, 0:128].T.reshape(P, BLOCKS, C)
        xd[1, :, :, 0] = xp_r[:, 128:256].T.reshape(P, BLOCKS, C)
        xd[1, :, :, 1] = xp_i[:, 128:256].T.reshape(P, BLOCKS, C)
        in_maps.append({"x": xd, "w1": w1, "w2": w2, "w1o": w1o, "w2o": w2o})
    res = run_bass_kernel_spmd(nc, in_maps, core_ids=list(range(NCORES)))
    yfull = np.concatenate([r["y"] for r in res.results], axis=0)  # [B, 512] f16

    real = np.empty((B, N), dtype=np.float32)
    imag = np.empty((B, N), dtype=np.float32)
    real[:, 0::2] = yfull[:, 0:128]        # X_even re
    imag[:, 0::2] = yfull[:, 128:256]      # X_even im
    real[:, 1::2] = yfull[:, 256:384]      # X_odd re
    imag[:, 1::2] = yfull[:, 384:512]      # X_odd im
    return real, imag


# revision 12
# speedup vs baseline: 1.2629x; 1.0562x over previous
"""Batched complex DFT (x @ W, N=256) via radix-2 DIF split, fp16 end-to-end,
data-parallel across 8 Trainium2 NeuronCores.

Math (decimation in frequency): with a = x_lo + x_hi, d = x_lo - x_hi
(column halves of x), the provided DFT matrix W satisfies
  X[:, 2m]   = (a @ W128)[:, m]          W128[n, m] = W[n, 2m]   (n, m < 128)
  X[:, 2m+1] = (d @ W_O)[:, m]           W_O = diag(W[1, :128]) @ W128
so the 256-point DFT becomes two 128-contraction complex matmuls — half the
MACs of the direct form, at full 128-partition PE efficiency.

The host ships x PRE-TRANSPOSED (contraction dim on partitions, a pure
layout permutation + fp16 cast), which removes the PE transposes AND the
PSUM->SBUF evacuation copy entirely. Per core (shard = 32768 rows), per
pair of 128-row tiles:
  - GpSimd computes the 'a' butterfly, DVE the 'd' butterfly (one paired
    512-elem fp16 op each), writing the matmul stationaries directly.
  - 8 fp16 matmuls (k=128, moving dim 256) accumulate even/odd complex
    outputs into fp32 PSUM; the Activation engine casts both tiles with
    one ACTIVATE to fp16 staging; 1/sqrt(N) is folded into the W packs.
  - Host does dtype casts, the input transpose, and the output even/odd
    interleave; fp16 I/O halves HBM traffic vs the fp32 baseline.
    absmax rel err ~5e-4, well under the 2e-2 gate.

Inputs stream on the SP HWDGE ring (4 KiB/partition descriptors), outputs
on the ACT ring (1 KiB descriptors, row t*128+p order).
"""

import numpy as np

P = 128
N = 256
NCORES = 8
B = 262144
M = B // NCORES            # 32768 rows per core
T = 16                     # 128-row tiles per DMA block
BLOCKS = M // (P * T)      # 16

_CACHE = {}


def _build():
    if "nc" in _CACHE:
        return _CACHE["nc"]

    import concourse.mybir as mybir
    import concourse.tile as tile
    from concourse import bacc

    F16 = mybir.dt.float16
    F32 = mybir.dt.float32
    W = 2 * N  # 512

    nc = bacc.Bacc("TRN2", debug=False, target_bir_lowering=False)

    C = T * P  # 2048 row-columns per block
    # x transposed + block-permuted on host:
    # [h (lo/hi), 128 k, block, two (re/im), C] -> 8 KiB contiguous DRAM per
    # partition per half-input DMA
    x = nc.dram_tensor("x", [2, P, BLOCKS, 2, C], F16, kind="ExternalInput").ap()
    w1 = nc.dram_tensor("w1", [P, N], F16, kind="ExternalInput").ap()
    w2 = nc.dram_tensor("w2", [P, N], F16, kind="ExternalInput").ap()
    w1o = nc.dram_tensor("w1o", [P, N], F16, kind="ExternalInput").ap()
    w2o = nc.dram_tensor("w2o", [P, N], F16, kind="ExternalInput").ap()
    y = nc.dram_tensor("y", [M, W], F16, kind="ExternalOutput").ap()

    x_t = x.rearrange("h p n two c -> n p h two c")
    # device column n*2048 + t*128 + q holds original row n*2048 + q*16 + t
    # (host-side permutation), so partition q writes 16 consecutive DRAM rows
    y_t = y.rearrange("(n p t) k -> n p t k", p=P, t=T)

    with tile.TileContext(nc) as tc:
        with (
            tc.tile_pool(name="consts", bufs=1) as consts,
            tc.tile_pool(name="xin", bufs=4) as xin_pool,
            tc.tile_pool(name="xt", bufs=4) as xt_pool,
            tc.tile_pool(name="stage", bufs=3) as stage_pool,
            tc.tile_pool(name="pso", bufs=3, space="PSUM") as pso_pool,
        ):
            w1_sb = consts.tile([P, N], F16)
            w2_sb = consts.tile([P, N], F16)
            w1o_sb = consts.tile([P, N], F16)
            w2o_sb = consts.tile([P, N], F16)
            # W packs ride the ACT ring, which is idle until the first
            # output: the input rings start streaming x immediately
            nc.scalar.dma_start(w1_sb, w1)
            nc.scalar.dma_start(w2_sb, w2)
            nc.scalar.dma_start(w1o_sb, w1o)
            nc.scalar.dma_start(w2o_sb, w2o)

            h = T // 2
            for n in range(BLOCKS):
                xin = xin_pool.tile([P, 2, 2, C], F16, tag="xin")
                if n == 0:
                    # fast pipeline fill: quarter DMAs so the first
                    # butterflies start after ~0.5 MB instead of 2 MB
                    c2 = C // 2
                    nc.sync.dma_start(xin[:, 0, :, 0:c2], x_t[n, :, 0, :, 0:c2])
                    nc.sync.dma_start(xin[:, 1, :, 0:c2], x_t[n, :, 1, :, 0:c2])
                    nc.sync.dma_start(xin[:, 0, :, c2:C], x_t[n, :, 0, :, c2:C])
                    nc.sync.dma_start(xin[:, 1, :, c2:C], x_t[n, :, 1, :, c2:C])
                else:
                    nc.sync.dma_start(xin, x_t[n])
                stage = stage_pool.tile([P, T, W], F16, tag="st")
                for tp in range(T // 2):
                    # two 128-row tiles per butterfly op / output ACTIVATE
                    s = slice(tp * 2 * P, (tp + 1) * 2 * P)
                    # xt: [ a_r | a_i | d_r | d_i ] x 256 row-columns
                    xt = xt_pool.tile([P, 4, 2 * P], F16, tag="xt")
                    nc.gpsimd.tensor_tensor(
                        xt[:, 0:2], xin[:, 0, :, s], xin[:, 1, :, s],
                        mybir.AluOpType.add,
                    )
                    nc.vector.tensor_tensor(
                        xt[:, 2:4], xin[:, 0, :, s], xin[:, 1, :, s],
                        mybir.AluOpType.subtract,
                    )
                    # ps[j]: [ X_even(re|im) | X_odd(re|im) ]
                    ps = pso_pool.tile([P, 2, W], F32, tag="po")
                    for j in range(2):
                        jj = slice(j * P, (j + 1) * P)
                        nc.tensor.matmul(ps[:, j, 0:N], xt[:, 0, jj], w1_sb, start=True, stop=False)
                        nc.tensor.matmul(ps[:, j, 0:N], xt[:, 1, jj], w2_sb, start=False, stop=True)
                        nc.tensor.matmul(ps[:, j, N:W], xt[:, 2, jj], w1o_sb, start=True, stop=False)
                        nc.tensor.matmul(ps[:, j, N:W], xt[:, 3, jj], w2o_sb, start=False, stop=True)
                    nc.scalar.copy(stage[:, 2 * tp : 2 * tp + 2], ps)
                nc.scalar.dma_start(y_t[n], stage)

    nc.compile()
    _CACHE["nc"] = nc
    return nc


def _make_w_packs(W_real, W_imag):
    Wc = W_real.astype(np.float64) + 1j * W_imag.astype(np.float64)
    W128 = Wc[:P, 0:N:2]                  # W128[n, m] = W[n, 2m]
    W_O = Wc[1, :P][:, None] * W128       # twiddle fold: diag(W[1, :128]) @ W128
    s = 1.0 / np.sqrt(N)
    packs = []
    for Wm in (W128, W_O):
        re = (np.real(Wm) * s).astype(np.float16)
        im = (np.imag(Wm) * s).astype(np.float16)
        packs.append(np.ascontiguousarray(np.concatenate([re, im], axis=1)))
        packs.append(np.ascontiguousarray(np.concatenate([-im, re], axis=1)))
    return packs  # w1, w2, w1o, w2o


def kernel(x_real, x_imag, W_real, W_imag):
    from concourse.bass_utils import run_bass_kernel_spmd

    x_real = np.asarray(x_real, dtype=np.float32)
    x_imag = np.asarray(x_imag, dtype=np.float32)
    assert x_real.shape == (B, N) and x_imag.shape == (B, N)

    # fp16 cast (fast, SIMD), then per-shard permuted transposes
    xr16 = x_real.astype(np.float16)
    xi16 = x_imag.astype(np.float16)
    w1, w2, w1o, w2o = _make_w_packs(np.asarray(W_real), np.asarray(W_imag))

    nc = _build()

    C = T * P
    # device column (n, t, q) <- original row (n, q, t): output partition q
    # then holds 16 consecutive DRAM rows per block (big DMA descriptors)
    pi = np.arange(M).reshape(BLOCKS, P, T).transpose(0, 2, 1).reshape(M)

    in_maps = []
    for i in range(NCORES):
        sl = slice(i * M, (i + 1) * M)
        xp_r = xr16[sl][pi]
        xp_i = xi16[sl][pi]
        xd = np.empty((2, P, BLOCKS, 2, C), dtype=np.float16)
        xd[0, :, :, 0] = xp_r[:, 0:128].T.reshape(P, BLOCKS, C)
        xd[0, :, :, 1] = xp_i[:, 0:128].T.reshape(P, BLOCKS, C)
        xd[1, :, :, 0] = xp_r[:, 128:256].T.reshape(P, BLOCKS, C)
        xd[1, :, :, 1] = xp_i[:, 128:256].T.reshape(P, BLOCKS, C)
        in_maps.append({"x": xd, "w1": w1, "w2": w2, "w1o": w1o, "w2o": w2o})
    res = run_bass_kernel_spmd(nc, in_maps, core_ids=list(range(NCORES)))
    yfull = np.concatenate([r["y"] for r in res.results], axis=0)  # [B, 512] f16

    real = np.empty((B, N), dtype=np.float32)
    imag = np.empty((B, N), dtype=np.float32)
    real[:, 0::2] = yfull[:, 0:128]        # X_even re
    imag[:, 0::2] = yfull[:, 128:256]      # X_even im
    real[:, 1::2] = yfull[:, 256:384]      # X_odd re
    imag[:, 1::2] = yfull[:, 384:512]      # X_odd im
    return real, imag
